# revision 1
# baseline (speedup 1.0000x reference)
"""Trainium2 Bass kernel for nn_Experts (topk_masking).

Math (reference):
  R = concat(h,us,ue) @ W_r.T + b_r                       [1,1,512]
  x = concat(u, R.broadcast)                              [1,S,1536]
  h1 = (x @ W_nn.T + b_nn).reshape(S,512,16)
  h2 = (x @ W_no.T + b_no).reshape(S,512,16) * noise
  g  = top2-masked softmax over experts of (h1+h2)
  e  = (x @ W_E.T + b_E).reshape(S,512,16)
  out = (g*e).mean(-1)                                    [1,S,512]

Sharding: the NE*DIM output-feature dim of the three projections is sharded
across 8 cores (64 dims x 16 experts each, contiguous feature slice). The
token-independent R-path is folded into a per-feature constant c[f] computed
once per core, so the per-token matmuls contract only over u's 1024 features.

Precision: gating matmuls use a 2-term fp32 split (11 explicit mantissa bits
+ residual) through the PE's float32r mode (verified: <=11-bit operands pass
through exactly), plus a bf16 cross-term; this lands the gating logits at
~fp32 accuracy so top-2 selection matches the fp32 reference. The e-matmul
runs in bf16 (smooth, no selection discontinuity).
"""
import numpy as np
import ml_dtypes

DIM = 512
NE = 16
S = 4096
KU = 2 * DIM        # u features = 1024
KR = DIM            # R features = 512
KX = 5 * DIM        # concat(h,us,ue) = 2560
NCORES = 8
DL = DIM // NCORES  # 64 dims per core
FL = DL * NE        # 1024 features per core
MCH = S // 128      # 32 token chunks

_MASK11 = np.uint32(0xFFFFF000)  # keep 11 explicit mantissa bits

TRACE = False
DEBUG = False
_CACHE = {}


def _trunc11(a):
    a = np.ascontiguousarray(a, dtype=np.float32)
    return (a.view(np.uint32) & _MASK11).view(np.float32)


def _build():
    import concourse.bass as bass
    import concourse.mybir as mybir
    import concourse.tile as tile
    from concourse import bacc
    from contextlib import ExitStack

    F32 = mybir.dt.float32
    F32R = mybir.dt.float32r
    BF16 = mybir.dt.bfloat16
    U32 = mybir.dt.uint32
    AX = mybir.AxisListType
    OP = mybir.AluOpType
    ACTF = mybir.ActivationFunctionType

    nc = bacc.Bacc("TRN2", target_bir_lowering=False, debug=False,
                   num_devices=NCORES)

    def dram(name, shape, dt, kind="ExternalInput"):
        return nc.dram_tensor(name, shape, dt, kind=kind)

    # per-core inputs (same names on every core; data differs per core)
    uhT = dram("uhT", [KU, S], F32R)
    ulT = dram("ulT", [KU, S], F32R)
    u8T = dram("u8T", [KU, S], BF16)
    whnnT = dram("whnnT", [KU, FL], F32R)
    whnoT = dram("whnoT", [KU, FL], F32R)
    wl8T = dram("wl8T", [KU, 2 * FL], BF16)   # [:, :FL]=nn resid, [:, FL:]=no resid
    we8T = dram("we8T", [KU, FL], BF16)
    noise_c = dram("noise_c", [S, FL], F32)
    hxf = dram("hxf", [KX], F32)
    wrT = dram("wrT", [KX, KR], F32)
    b_r = dram("b_r", [KR], F32)
    wRh_nn = dram("wRh_nn", [KR, FL], F32R)
    wRl_nn = dram("wRl_nn", [KR, FL], F32R)
    wRh_no = dram("wRh_no", [KR, FL], F32R)
    wRl_no = dram("wRl_no", [KR, FL], F32R)
    wR_E = dram("wR_E", [KR, FL], F32R)
    bias_c = dram("bias_c", [3 * FL], F32)
    out_c = dram("out_c", [S, DL], F32, kind="ExternalOutput")
    dbg = {}
    if DEBUG:
        for nm in ["h1", "h2", "e", "m", "q", "mask"]:
            dbg[nm] = dram("dbg_" + nm, [128, FL], F32, kind="ExternalOutput")
        for nm in ["v1", "v2", "s"]:
            dbg[nm] = dram("dbg_" + nm, [128, DL], F32, kind="ExternalOutput")
        dbg["cc"] = dram("dbg_cc", [2, 3 * FL], F32, kind="ExternalOutput")
        dbg["R"] = dram("dbg_R", [128, 4], F32, kind="ExternalOutput")

    with tile.TileContext(nc) as tc, ExitStack() as ctx:
        wpool = ctx.enter_context(tc.tile_pool(name="w", bufs=1))

        # resident weights (one big DMA each)
        whnn_t = wpool.tile([128, 8, FL], F32R)
        whno_t = wpool.tile([128, 8, FL], F32R)
        wl8_t = wpool.tile([128, 8, 2 * FL], BF16)
        we8_t = wpool.tile([128, 8, FL], BF16)
        nc.sync.dma_start(whnn_t[:], whnnT.ap().rearrange("(kc p) f -> p kc f", p=128))
        nc.sync.dma_start(whno_t[:], whnoT.ap().rearrange("(kc p) f -> p kc f", p=128))
        nc.sync.dma_start(wl8_t[:], wl8T.ap().rearrange("(kc p) f -> p kc f", p=128))
        nc.sync.dma_start(we8_t[:], we8T.ap().rearrange("(kc p) f -> p kc f", p=128))

        # survives the whole kernel: bias/R constant rows + ones for the K=2 matmul
        ccsb = wpool.tile([2, 3 * FL], F32R)
        onesf = wpool.tile([2, 128], F32)
        nc.vector.memset(onesf[:], 1.0)
        ones2 = wpool.tile([2, 128], F32R)
        nc.vector.tensor_copy(ones2[:], onesf[:])

        # ---------------- stage 0: R then c ----------------
        with ExitStack() as s0:
            s0sb = s0.enter_context(tc.tile_pool(name="s0sb", bufs=1))
            s0rot = s0.enter_context(tc.tile_pool(name="s0rot", bufs=4))
            s0ps = s0.enter_context(tc.tile_pool(name="s0ps", bufs=1, space="PSUM"))

            hx_t = s0sb.tile([128, 20], F32)
            nc.sync.dma_start(hx_t[:], hxf.ap().rearrange("(kc p) -> p kc", p=128))

            # R = hx @ W_r.T with W_r stationary: out lands as [128, 4]
            # across partitions directly (R[mo*128+p] = psR[p, mo]).
            # NOTE: start=True clears has_written for the whole PSUM bank, so
            # each mo's accumulation chain needs its own bank.
            psR = []
            for mo in range(4):
                psR_mo = s0ps.tile([128, 1], F32, tag=f"psR{mo}")
                psR.append(psR_mo)
            for kc in range(20):
                ksl = slice(kc * 128, (kc + 1) * 128)
                wr_ch = s0rot.tile([128, KR], F32, tag="rotf")
                nc.sync.dma_start(wr_ch[:], wrT.ap()[ksl, :])
                for mo in range(4):
                    msl = slice(mo * 128, (mo + 1) * 128)
                    nc.tensor.matmul(psR[mo][:], wr_ch[:, msl],
                                     hx_t[:, kc:kc + 1],
                                     start=(kc == 0), stop=(kc == 19))

            brt = s0sb.tile([128, 4], F32)
            nc.sync.dma_start(brt[:], b_r.ap().rearrange("(mo p) -> p mo", p=128))
            Rcol = s0sb.tile([128, 4], F32)
            for mo in range(4):
                nc.vector.tensor_add(Rcol[:, mo:mo + 1], psR[mo][:],
                                     brt[:, mo:mo + 1])

            Rh = s0sb.tile([128, 4], F32)
            nc.vector.tensor_scalar(Rh[:].bitcast(U32), Rcol[:].bitcast(U32),
                                    int(_MASK11), None, OP.bitwise_and)
            Rl = s0sb.tile([128, 4], F32)
            nc.vector.tensor_sub(Rl[:], Rcol[:], Rh[:])
            # broadcast along the stationary M dim (value replicated per token)
            Rbch = s0sb.tile([128, 4, 128], F32R)
            nc.vector.tensor_copy(Rbch[:], Rh[:].broadcast_to([128, 4, 128]))
            Rbcl = s0sb.tile([128, 4, 128], F32R)
            nc.vector.tensor_copy(Rbcl[:], Rl[:].broadcast_to([128, 4, 128]))
            if DEBUG:
                nc.sync.dma_start(dbg["R"].ap(), Rcol[:])

            # c pieces: piece 0 -> c_nn, 1 -> c_no, 2 -> c_E (each FL wide)
            # biasb2 doubles as the c+bias staging buffer (updated in place);
            # all DVE work stays on partition 0 (engines need lane-0 alignment)
            biasb2 = s0sb.tile([1, 3 * FL], F32)
            nc.sync.dma_start(biasb2[:],
                              bias_c.ap().rearrange("(o f) -> o f", o=1))
            cpsum = s0ps.tile([128, FL], F32, tag="cps")
            pieces = [(wRh_nn, wRl_nn), (wRh_no, wRl_no), (wR_E, None)]
            for pi, (wh_d, wl_d) in enumerate(pieces):
                for kc in range(4):
                    ksl = slice(kc * 128, (kc + 1) * 128)
                    for half in range(2):
                        fsl = slice(half * 512, (half + 1) * 512)
                        whch = s0rot.tile([128, 512], F32R, tag="rot")
                        nc.sync.dma_start(whch[:], wh_d.ap()[ksl, fsl])
                        nc.tensor.matmul(cpsum[:, fsl], Rbch[:, kc, :], whch[:],
                                         start=(kc == 0), stop=False)
                        nc.tensor.matmul(cpsum[:, fsl], Rbcl[:, kc, :], whch[:],
                                         start=False, stop=False)
                        if wl_d is not None:
                            wlch = s0rot.tile([128, 512], F32R, tag="rot")
                            nc.sync.dma_start(wlch[:], wl_d.ap()[ksl, fsl])
                            nc.tensor.matmul(cpsum[:, fsl], Rbch[:, kc, :],
                                             wlch[:], start=False,
                                             stop=(kc == 3))
                        elif kc == 3:
                            nc.tensor.matmul(cpsum[:, fsl], Rbcl[:, kc, :],
                                             whch[:], start=False, stop=True)
                psl = slice(pi * FL, (pi + 1) * FL)
                nc.vector.tensor_add(biasb2[0:1, psl], cpsum[0:1, :],
                                     biasb2[0:1, psl])

            # split c into 11-bit head + residual, round both to f32r on
            # partition 0, then DMA into the two rows of ccsb
            cht = s0sb.tile([1, 3 * FL], F32)
            nc.vector.tensor_scalar(cht[0:1, :].bitcast(U32),
                                    biasb2[0:1, :].bitcast(U32),
                                    int(_MASK11), None, OP.bitwise_and)
            clt = s0sb.tile([1, 3 * FL], F32)
            nc.vector.tensor_sub(clt[0:1, :], biasb2[0:1, :], cht[0:1, :])
            chr_ = s0sb.tile([1, 3 * FL], F32R)
            nc.vector.tensor_copy(chr_[0:1, :], cht[0:1, :])
            clr_ = s0sb.tile([1, 3 * FL], F32R)
            nc.vector.tensor_copy(clr_[0:1, :], clt[0:1, :])
            nc.sync.dma_start(ccsb[0:1, :], chr_[0:1, :])
            nc.sync.dma_start(ccsb[1:2, :], clr_[0:1, :])
            if DEBUG:
                nc.sync.dma_start(dbg["cc"].ap()[0:1, :], cht[0:1, :])
                nc.sync.dma_start(dbg["cc"].ap()[1:2, :], clt[0:1, :])

        # ---------------- main loop over 32 token chunks ----------------
        spool = ctx.enter_context(tc.tile_pool(name="stream", bufs=2))
        epool = ctx.enter_context(tc.tile_pool(name="epi", bufs=1))
        mpsum = ctx.enter_context(tc.tile_pool(name="mps", bufs=1, space="PSUM"))

        uhT_r = uhT.ap().rearrange("(kc p) t -> p kc t", p=128)
        ulT_r = ulT.ap().rearrange("(kc p) t -> p kc t", p=128)
        u8T_r = u8T.ap().rearrange("(kc p) t -> p kc t", p=128)

        for m in range(MCH):
            tsl = slice(m * 128, (m + 1) * 128)
            xh_t = spool.tile([128, 8, 128], F32R, tag="xh")
            xl_t = spool.tile([128, 8, 128], F32R, tag="xl")
            x8_t = spool.tile([128, 8, 128], BF16, tag="x8")
            nz_t = spool.tile([128, FL], F32, tag="nz")
            nc.sync.dma_start(xh_t[:], uhT_r[:, :, tsl])
            nc.sync.dma_start(xl_t[:], ulT_r[:, :, tsl])
            nc.sync.dma_start(x8_t[:], u8T_r[:, :, tsl])
            nc.sync.dma_start(nz_t[:], noise_c.ap()[tsl, :])

            h1p = mpsum.tile([128, FL], F32, tag="h1")
            h2p = mpsum.tile([128, FL], F32, tag="h2")
            ep = mpsum.tile([128, FL], F32, tag="e")

            for k in range(8):
                lh = xh_t[:, k, :]
                ll = xl_t[:, k, :]
                l8 = x8_t[:, k, :]
                st = (k == 0)
                for half in range(2):
                    fsl = slice(half * 512, (half + 1) * 512)
                    # stationary xh: main gating terms
                    nc.tensor.matmul(h2p[:, fsl], lh, whno_t[:, k, fsl],
                                     start=st, stop=False)
                    nc.tensor.matmul(h1p[:, fsl], lh, whnn_t[:, k, fsl],
                                     start=st, stop=False)
                for half in range(2):
                    fsl = slice(half * 512, (half + 1) * 512)
                    # stationary xl: residual-x terms
                    nc.tensor.matmul(h2p[:, fsl], ll, whno_t[:, k, fsl],
                                     start=False, stop=False)
                    nc.tensor.matmul(h1p[:, fsl], ll, whnn_t[:, k, fsl],
                                     start=False, stop=False)
                for half in range(2):
                    fsl = slice(half * 512, (half + 1) * 512)
                    fsl_no = slice(FL + half * 512, FL + (half + 1) * 512)
                    # stationary x8 (bf16): residual-W cross terms + e matmul
                    nc.tensor.matmul(h2p[:, fsl], l8, wl8_t[:, k, fsl_no],
                                     start=False, stop=False)
                    nc.tensor.matmul(h1p[:, fsl], l8, wl8_t[:, k, fsl],
                                     start=False, stop=False)
                    nc.tensor.matmul(ep[:, fsl], l8, we8_t[:, k, fsl],
                                     start=st, stop=False)

            # bias + R-path constant via K=2 ones-matmul (rows: c_head, c_resid)
            for half in range(2):
                fsl = slice(half * 512, (half + 1) * 512)
                nc.tensor.matmul(h1p[:, fsl], ones2[:], ccsb[:, fsl],
                                 start=False, stop=True)
                nc.tensor.matmul(h2p[:, fsl], ones2[:],
                                 ccsb[:, FL + half * 512:FL + (half + 1) * 512],
                                 start=False, stop=True)
                nc.tensor.matmul(ep[:, fsl], ones2[:],
                                 ccsb[:, 2 * FL + half * 512:2 * FL + (half + 1) * 512],
                                 start=False, stop=True)

            # ---------------- epilogue ----------------
            if DEBUG and m == 0:
                for nm, src in [("h1", h1p), ("h2", h2p), ("e", ep)]:
                    dtmp = epool.tile([128, FL], F32, tag="dbg" + nm)
                    nc.scalar.copy(dtmp[:], src[:])
                    nc.sync.dma_start(dbg[nm].ap(), dtmp[:])
            t_t = epool.tile([128, FL], F32, tag="t")
            nc.vector.tensor_mul(t_t[:], h2p[:], nz_t[:])
            m_t = epool.tile([128, FL], F32, tag="m")
            nc.vector.tensor_add(m_t[:], t_t[:], h1p[:])

            mg = m_t[:].rearrange("p (d e) -> p d e", e=NE)
            v1 = epool.tile([128, DL], F32, tag="v1")
            nc.vector.tensor_reduce(v1[:], mg, AX.X, op=OP.max)
            eq1 = epool.tile([128, FL], F32, tag="eq1")
            nc.vector.tensor_tensor(eq1[:].rearrange("p (d e) -> p d e", e=NE),
                                    mg, v1[:].broadcast_to([128, DL, NE]),
                                    OP.is_equal)
            m2 = epool.tile([128, FL], F32, tag="m2")
            nc.vector.scalar_tensor_tensor(m2[:], eq1[:], -1e30, m_t[:],
                                           OP.mult, OP.add)
            v2 = epool.tile([128, DL], F32, tag="v2")
            nc.vector.tensor_reduce(v2[:], m2[:].rearrange("p (d e) -> p d e", e=NE),
                                    AX.X, op=OP.max)
            mask = epool.tile([128, FL], F32, tag="mask")
            nc.vector.tensor_tensor(mask[:].rearrange("p (d e) -> p d e", e=NE),
                                    mg, v2[:].broadcast_to([128, DL, NE]),
                                    OP.is_ge)
            q = epool.tile([128, FL], F32, tag="q")
            nc.scalar.activation(q[:], m_t[:], ACTF.Exp)

            t1 = epool.tile([128, FL], F32, tag="t1")
            nc.vector.tensor_mul(t1[:], mask[:], ep[:])
            t2 = epool.tile([128, FL], F32, tag="t2")
            nc.vector.tensor_mul(t2[:], t1[:], q[:])
            s_t = epool.tile([128, DL], F32, tag="s")
            nc.vector.tensor_reduce(s_t[:], t2[:].rearrange("p (d e) -> p d e", e=NE),
                                    AX.X, op=OP.add)

            if DEBUG and m == 0:
                for nm, src in [("m", m_t), ("q", q), ("mask", mask)]:
                    nc.sync.dma_start(dbg[nm].ap(), src[:])
                for nm, src in [("v1", v1), ("v2", v2), ("s", s_t)]:
                    nc.sync.dma_start(dbg[nm].ap(), src[:])
            ev12 = epool.tile([128, 2 * DL], F32, tag="ev12")
            nc.scalar.activation(ev12[:, :DL], v1[:], ACTF.Exp)
            nc.scalar.activation(ev12[:, DL:], v2[:], ACTF.Exp)
            z_t = epool.tile([128, DL], F32, tag="z")
            nc.vector.tensor_add(z_t[:], ev12[:, :DL], ev12[:, DL:])
            r_t = epool.tile([128, DL], F32, tag="r")
            nc.vector.reciprocal(r_t[:], z_t[:])
            o_t = epool.tile([128, DL], F32, tag="o")
            nc.vector.scalar_tensor_tensor(o_t[:], s_t[:], 1.0 / NE, r_t[:],
                                           OP.mult, OP.mult)
            nc.sync.dma_start(out_c.ap()[tsl, :], o_t[:])

    nc.compile()
    return nc


def _get_program():
    if "nc" not in _CACHE:
        _CACHE["nc"] = _build()
    return _CACHE["nc"]


def kernel(h, us, ue, u, noise, W_nn, b_nn, W_no, b_no, W_E, b_E, W_r, b_r):
    from concourse.bass_utils import run_bass_kernel_spmd

    f32 = np.float32
    bf16 = ml_dtypes.bfloat16
    u2 = np.ascontiguousarray(np.asarray(u, dtype=f32).reshape(S, KU))
    uh = _trunc11(u2)
    ul = (u2 - uh).astype(f32)
    uhT = np.ascontiguousarray(uh.T)
    ulT = np.ascontiguousarray(ul.T)
    u8T = np.ascontiguousarray(u2.T.astype(bf16))

    hx = np.concatenate([np.asarray(h, dtype=f32).ravel(),
                         np.asarray(us, dtype=f32).ravel(),
                         np.asarray(ue, dtype=f32).ravel()]).astype(f32)
    W_r = np.asarray(W_r, dtype=f32)
    wrT = np.ascontiguousarray(W_r.T)
    b_r = np.ascontiguousarray(np.asarray(b_r, dtype=f32))

    W_nn = np.asarray(W_nn, dtype=f32)
    W_no = np.asarray(W_no, dtype=f32)
    W_E = np.asarray(W_E, dtype=f32)
    b_nn = np.asarray(b_nn, dtype=f32)
    b_no = np.asarray(b_no, dtype=f32)
    b_E = np.asarray(b_E, dtype=f32)
    noise4 = np.asarray(noise, dtype=f32).reshape(S, DIM, NE)

    in_maps = []
    for c in range(NCORES):
        fsl = slice(c * FL, (c + 1) * FL)
        wnn_u = W_nn[fsl, :KU]
        wno_u = W_no[fsl, :KU]
        wE_u = W_E[fsl, :KU]
        wnn_h = _trunc11(wnn_u)
        wno_h = _trunc11(wno_u)
        wl8 = np.concatenate([(wnn_u - wnn_h).T.astype(bf16),
                              (wno_u - wno_h).T.astype(bf16)], axis=1)
        im = {
            "uhT": uhT, "ulT": ulT, "u8T": u8T,
            "whnnT": np.ascontiguousarray(wnn_h.T),
            "whnoT": np.ascontiguousarray(wno_h.T),
            "wl8T": np.ascontiguousarray(wl8),
            "we8T": np.ascontiguousarray(wE_u.T.astype(bf16)),
            "noise_c": np.ascontiguousarray(
                noise4[:, c * DL:(c + 1) * DL, :].reshape(S, FL)),
            "hxf": hx, "wrT": wrT, "b_r": b_r,
            "wRh_nn": np.ascontiguousarray(_trunc11(W_nn[fsl, KU:]).T),
            "wRl_nn": np.ascontiguousarray(
                (W_nn[fsl, KU:] - _trunc11(W_nn[fsl, KU:])).T.astype(f32)),
            "wRh_no": np.ascontiguousarray(_trunc11(W_no[fsl, KU:]).T),
            "wRl_no": np.ascontiguousarray(
                (W_no[fsl, KU:] - _trunc11(W_no[fsl, KU:])).T.astype(f32)),
            "wR_E": np.ascontiguousarray(W_E[fsl, KU:].T.astype(f32)),
            "bias_c": np.concatenate([b_nn[fsl], b_no[fsl], b_E[fsl]]).astype(f32),
        }
        in_maps.append(im)

    nc = _get_program()
    res = run_bass_kernel_spmd(nc, in_maps, core_ids=list(range(NCORES)),
                               trace=TRACE)
    _CACHE["last_results"] = res
    out = np.empty((1, S, DIM), dtype=f32)
    for c in range(NCORES):
        out[0, :, c * DL:(c + 1) * DL] = res.results[c]["out_c"]
    return out



# revision 2
# speedup vs baseline: 1.8271x; 1.8271x over previous
"""Trainium2 Bass kernel for nn_Experts (topk_masking).

Math (reference):
  R = concat(h,us,ue) @ W_r.T + b_r                       [1,1,512]
  x = concat(u, R.broadcast)                              [1,S,1536]
  h1 = (x @ W_nn.T + b_nn).reshape(S,512,16)
  h2 = (x @ W_no.T + b_no).reshape(S,512,16) * noise
  g  = top2-masked softmax over experts of (h1+h2)
  e  = (x @ W_E.T + b_E).reshape(S,512,16)
  out = (g*e).mean(-1)                                    [1,S,512]

Sharding: the NE*DIM output-feature dim of the three projections is sharded
across 8 cores (64 dims x 16 experts each, contiguous feature slice). The
token-independent R-path is computed on the host in fp64 and folded into a
per-feature constant c[f]; the device contracts only over u's 1024 features.

Precision: gating logits accumulate in one PSUM chain at scale 2^18:
  fp16(x) @ fp16(W*2^18)                 main term (<=11-bit products, exact)
  e4m3(xl*2^14) @ e4m3(wA*2^4)           x-residual term, fp8 DoubleRow
  e4m3(x*2^3) @ e4m3(wl*2^15)            W-residual term, fp8 DoubleRow
  + c*2^18 via a K=2 f32r ones-matmul (trunc11 2-row split)
The 2^-18 descale folds into exp()'s scale operand; max/compare ops are
scale-invariant. This lands gating logits at ~2^-16 relative accuracy so
top-2 selection matches the fp32 reference. The e-matmul runs in fp16.
"""
import numpy as np
import ml_dtypes

DIM = 512
NE = 16
S = 4096
KU = 2 * DIM        # u features = 1024
NCORES = 8
DL = DIM // NCORES  # 64 dims per core
FL = DL * NE        # 1024 features per core
MCH = S // 128      # 32 token chunks

SC = 2.0 ** 18      # gating PSUM scale
ISC = 2.0 ** -18

_MASK11 = np.uint32(0xFFFFF000)  # keep 11 explicit mantissa bits

TRACE = False
_CACHE = {}


def _trunc11(a):
    a = np.ascontiguousarray(a, dtype=np.float32)
    return (a.view(np.uint32) & _MASK11).view(np.float32)


def _build():
    import concourse.bass as bass
    import concourse.mybir as mybir
    import concourse.tile as tile
    from concourse import bacc
    from contextlib import ExitStack

    F32 = mybir.dt.float32
    F32R = mybir.dt.float32r
    F16 = mybir.dt.float16
    F8 = mybir.dt.float8e4
    AX = mybir.AxisListType
    OP = mybir.AluOpType
    ACTF = mybir.ActivationFunctionType
    DR = mybir.MatmulPerfMode.DoubleRow

    nc = bacc.Bacc("TRN2", target_bir_lowering=False, debug=False,
                   num_devices=NCORES)

    def dram(name, shape, dt, kind="ExternalInput"):
        return nc.dram_tensor(name, shape, dt, kind=kind)

    # per-core inputs (same names on every core; data differs per core)
    xhT = dram("xhT", [128, 8, S], F16)           # fp16(u), k = kc*128+p
    xl8T = dram("xl8T", [128, 2, 4, S], F8)       # e4m3(xl*2^14), k=256j+128i+p
    x88T = dram("x88T", [128, 2, 4, S], F8)       # e4m3(u*2^3)
    noise_c = dram("noise_c", [S, FL], F32)
    whnn16 = dram("whnn16", [128, 8, FL], F16)    # fp16(W_nn.T*2^18)
    whno16 = dram("whno16", [128, 8, FL], F16)
    weh16 = dram("weh16", [128, 8, FL], F16)      # fp16(W_E.T)
    wh8nn = dram("wh8nn", [128, 2, 4, FL], F8)    # e4m3(wA_nn.T*2^4)
    wl8nn = dram("wl8nn", [128, 2, 4, FL], F8)    # e4m3(wl_nn.T*2^15)
    wh8no = dram("wh8no", [128, 2, 4, FL], F8)
    wl8no = dram("wl8no", [128, 2, 4, FL], F8)
    ccd = dram("ccd", [2, 3 * FL], F32R)          # trunc11 rows of c constants
    out_c = dram("out_c", [S, DL], F32, kind="ExternalOutput")

    with tile.TileContext(nc) as tc, ExitStack() as ctx:
        wpool = ctx.enter_context(tc.tile_pool(name="w", bufs=1))

        # resident weights (one big DMA each); e-weights first so the PE can
        # start chunk 0's e-phase as early as possible
        weh_t = wpool.tile([128, 8, FL], F16)
        whno_t = wpool.tile([128, 8, FL], F16)
        wh8no_t = wpool.tile([128, 2, 4, FL], F8)
        wl8no_t = wpool.tile([128, 2, 4, FL], F8)
        whnn_t = wpool.tile([128, 8, FL], F16)
        wh8nn_t = wpool.tile([128, 2, 4, FL], F8)
        wl8nn_t = wpool.tile([128, 2, 4, FL], F8)
        ccsb = wpool.tile([2, 3 * FL], F32R)
        nc.sync.dma_start(weh_t[:], weh16.ap())
        nc.sync.dma_start(ccsb[:], ccd.ap())
        nc.sync.dma_start(whno_t[:], whno16.ap())
        nc.sync.dma_start(wh8no_t[:], wh8no.ap())
        nc.sync.dma_start(wl8no_t[:], wl8no.ap())
        nc.sync.dma_start(whnn_t[:], whnn16.ap())
        nc.sync.dma_start(wh8nn_t[:], wh8nn.ap())
        nc.sync.dma_start(wl8nn_t[:], wl8nn.ap())

        onesf = wpool.tile([2, 128], F32)
        nc.vector.memset(onesf[:], 1.0)
        ones2 = wpool.tile([2, 128], F32R)
        nc.vector.tensor_copy(ones2[:], onesf[:])

        spool = ctx.enter_context(tc.tile_pool(name="stream", bufs=2))
        epool = ctx.enter_context(tc.tile_pool(name="epi", bufs=2))
        phpool = ctx.enter_context(tc.tile_pool(name="phps", bufs=1,
                                                space="PSUM"))
        pepool = ctx.enter_context(tc.tile_pool(name="peps", bufs=2,
                                                space="PSUM"))

        for m in range(MCH):
            tsl = slice(m * 128, (m + 1) * 128)
            xh_t = spool.tile([128, 8, 128], F16, tag="xh")
            xl8_t = spool.tile([128, 2, 4, 128], F8, tag="xl8")
            x88_t = spool.tile([128, 2, 4, 128], F8, tag="x88")
            nz_t = spool.tile([128, FL], F32, tag="nz")
            nc.sync.dma_start(xh_t[:], xhT.ap()[:, :, tsl])
            nc.sync.dma_start(xl8_t[:], xl8T.ap()[:, :, :, tsl])
            nc.sync.dma_start(x88_t[:], x88T.ap()[:, :, :, tsl])
            nc.sync.dma_start(nz_t[:], noise_c.ap()[tsl, :])

            ep = pepool.tile([128, FL], F32, tag="e")
            h1p = phpool.tile([128, FL], F32, tag="h1")
            h2p = phpool.tile([128, FL], F32, tag="h2")

            # ---- e phase (fp16) -------------------------------------------
            for k in range(8):
                st = (k == 0)
                for half in range(2):
                    fsl = slice(half * 512, (half + 1) * 512)
                    nc.tensor.matmul(ep[:, fsl], xh_t[:, k, :],
                                     weh_t[:, k, fsl], start=st, stop=False)
            for half in range(2):
                fsl = slice(half * 512, (half + 1) * 512)
                nc.tensor.matmul(ep[:, fsl], ones2[:],
                                 ccsb[:, 2 * FL + half * 512:
                                      2 * FL + (half + 1) * 512],
                                 start=False, stop=True)
            # evacuate e early on the scalar engine to free its PSUM banks
            es = epool.tile([128, FL], F32, tag="es")
            nc.scalar.copy(es[:], ep[:])

            # ---- h2 phase: fp16 mains + fp8 DoubleRow residuals -----------
            for k in range(8):
                st = (k == 0)
                for half in range(2):
                    fsl = slice(half * 512, (half + 1) * 512)
                    nc.tensor.matmul(h2p[:, fsl], xh_t[:, k, :],
                                     whno_t[:, k, fsl], start=st, stop=False)
            for j in range(4):
                for half in range(2):
                    fsl = slice(half * 512, (half + 1) * 512)
                    nc.tensor.matmul(h2p[:, fsl], xl8_t[:, :, j, :],
                                     wh8no_t[:, :, j, fsl],
                                     start=False, stop=False, perf_mode=DR)
                    nc.tensor.matmul(h2p[:, fsl], x88_t[:, :, j, :],
                                     wl8no_t[:, :, j, fsl],
                                     start=False, stop=False, perf_mode=DR)
            for half in range(2):
                fsl = slice(half * 512, (half + 1) * 512)
                nc.tensor.matmul(h2p[:, fsl], ones2[:],
                                 ccsb[:, FL + half * 512:FL + (half + 1) * 512],
                                 start=False, stop=True)
            t_t = epool.tile([128, FL], F32, tag="t")
            nc.vector.tensor_mul(t_t[:], h2p[:], nz_t[:])

            # ---- h1 phase -------------------------------------------------
            for k in range(8):
                st = (k == 0)
                for half in range(2):
                    fsl = slice(half * 512, (half + 1) * 512)
                    nc.tensor.matmul(h1p[:, fsl], xh_t[:, k, :],
                                     whnn_t[:, k, fsl], start=st, stop=False)
            for j in range(4):
                for half in range(2):
                    fsl = slice(half * 512, (half + 1) * 512)
                    nc.tensor.matmul(h1p[:, fsl], xl8_t[:, :, j, :],
                                     wh8nn_t[:, :, j, fsl],
                                     start=False, stop=False, perf_mode=DR)
                    nc.tensor.matmul(h1p[:, fsl], x88_t[:, :, j, :],
                                     wl8nn_t[:, :, j, fsl],
                                     start=False, stop=False, perf_mode=DR)
            for half in range(2):
                fsl = slice(half * 512, (half + 1) * 512)
                nc.tensor.matmul(h1p[:, fsl], ones2[:],
                                 ccsb[:, half * 512:(half + 1) * 512],
                                 start=False, stop=True)
            m_t = epool.tile([128, FL], F32, tag="m")
            nc.vector.tensor_add(m_t[:], t_t[:], h1p[:])

            # ---- top-2 masked softmax epilogue (values at scale 2^18) -----
            mg = m_t[:].rearrange("p (d e) -> p d e", e=NE)
            v1 = epool.tile([128, DL], F32, tag="v1")
            nc.vector.tensor_reduce(v1[:], mg, AX.X, op=OP.max)
            eq1 = epool.tile([128, FL], F32, tag="eq1")
            nc.vector.tensor_tensor(eq1[:].rearrange("p (d e) -> p d e", e=NE),
                                    mg, v1[:].broadcast_to([128, DL, NE]),
                                    OP.is_equal)
            m2 = epool.tile([128, FL], F32, tag="m2")
            nc.vector.scalar_tensor_tensor(m2[:], eq1[:], -1e30, m_t[:],
                                           OP.mult, OP.add)
            v2 = epool.tile([128, DL], F32, tag="v2")
            nc.vector.tensor_reduce(v2[:], m2[:].rearrange("p (d e) -> p d e",
                                                           e=NE),
                                    AX.X, op=OP.max)
            mask = epool.tile([128, FL], F32, tag="mask")
            nc.vector.tensor_tensor(mask[:].rearrange("p (d e) -> p d e", e=NE),
                                    mg, v2[:].broadcast_to([128, DL, NE]),
                                    OP.is_ge)
            q = epool.tile([128, FL], F32, tag="q")
            nc.scalar.activation(q[:], m_t[:], ACTF.Exp, scale=ISC)

            t1 = epool.tile([128, FL], F32, tag="t1")
            nc.vector.tensor_mul(t1[:], mask[:], es[:])
            t2 = epool.tile([128, FL], F32, tag="t2")
            nc.vector.tensor_mul(t2[:], t1[:], q[:])
            s_t = epool.tile([128, DL], F32, tag="s")
            nc.vector.tensor_reduce(s_t[:],
                                    t2[:].rearrange("p (d e) -> p d e", e=NE),
                                    AX.X, op=OP.add)

            ev12 = epool.tile([128, 2 * DL], F32, tag="ev12")
            nc.scalar.activation(ev12[:, :DL], v1[:], ACTF.Exp, scale=ISC)
            nc.scalar.activation(ev12[:, DL:], v2[:], ACTF.Exp, scale=ISC)
            z_t = epool.tile([128, DL], F32, tag="z")
            nc.vector.tensor_add(z_t[:], ev12[:, :DL], ev12[:, DL:])
            r_t = epool.tile([128, DL], F32, tag="r")
            nc.vector.reciprocal(r_t[:], z_t[:])
            o_t = epool.tile([128, DL], F32, tag="o")
            nc.vector.scalar_tensor_tensor(o_t[:], s_t[:], 1.0 / NE, r_t[:],
                                           OP.mult, OP.mult)
            nc.sync.dma_start(out_c.ap()[tsl, :], o_t[:])

    nc.compile()
    return nc


def _get_program():
    if "nc" not in _CACHE:
        _CACHE["nc"] = _build()
    return _CACHE["nc"]


def _prep_shared(u):
    f32 = np.float32
    E4 = ml_dtypes.float8_e4m3
    u2 = np.ascontiguousarray(np.asarray(u, dtype=f32).reshape(S, KU))
    xh16 = u2.astype(np.float16)                      # [S, K]
    xl = (u2 - xh16.astype(f32)).astype(f32)

    # main layout [p, kc, t]: k = kc*128 + p
    xhT = np.ascontiguousarray(xh16.T.reshape(8, 128, S).transpose(1, 0, 2))
    # DoubleRow layout [p, i, j, t]: k = j*256 + i*128 + p
    xl8T = np.ascontiguousarray(
        (xl.T * f32(2.0 ** 14)).astype(E4)
        .reshape(4, 2, 128, S).transpose(2, 1, 0, 3))
    x88T = np.ascontiguousarray(
        (u2.T * f32(2.0 ** 3)).astype(E4)
        .reshape(4, 2, 128, S).transpose(2, 1, 0, 3))
    return u2, xhT, xl8T, x88T


def _prep_gating_w(Wu, scale):
    # Wu: [FL, KU] fp32 feature-slice of a gating projection (u-part)
    f32 = np.float32
    E4 = ml_dtypes.float8_e4m3
    WuT = np.ascontiguousarray(Wu.T.astype(f32))      # [K, F]
    wh16 = (WuT * f32(scale)).astype(np.float16)
    wA = (wh16.astype(f32) * f32(1.0 / scale)).astype(f32)
    wl = (WuT - wA).astype(f32)
    w16 = np.ascontiguousarray(wh16.reshape(8, 128, FL).transpose(1, 0, 2))
    wh8 = np.ascontiguousarray(
        (wA * f32(2.0 ** 4)).astype(E4)
        .reshape(4, 2, 128, FL).transpose(2, 1, 0, 3))
    wl8 = np.ascontiguousarray(
        (wl * f32(2.0 ** 15)).astype(E4)
        .reshape(4, 2, 128, FL).transpose(2, 1, 0, 3))
    return w16, wh8, wl8


def kernel(h, us, ue, u, noise, W_nn, b_nn, W_no, b_no, W_E, b_E, W_r, b_r):
    from concourse.bass_utils import run_bass_kernel_spmd

    f32 = np.float32
    u2, xhT, xl8T, x88T = _prep_shared(u)

    # host R-path in fp64 (token-independent, ~4 MFLOP)
    hx = np.concatenate([np.asarray(h, dtype=f32).ravel(),
                         np.asarray(us, dtype=f32).ravel(),
                         np.asarray(ue, dtype=f32).ravel()]).astype(np.float64)
    R = hx @ np.asarray(W_r, dtype=np.float64).T + np.asarray(
        b_r, dtype=np.float64)                        # [512]

    W_nn = np.asarray(W_nn, dtype=f32)
    W_no = np.asarray(W_no, dtype=f32)
    W_E = np.asarray(W_E, dtype=f32)
    noise4 = np.asarray(noise, dtype=f32).reshape(S, DIM, NE)

    def cc_rows(W, b, fsl, scale):
        c = (np.asarray(b, np.float64)[fsl]
             + R @ np.asarray(W, np.float64)[fsl, KU:].T) * scale
        c32 = c.astype(f32)
        ch = _trunc11(c32)
        cl = _trunc11((c32.astype(np.float64) - ch).astype(f32))
        return ch, cl

    in_maps = []
    for c in range(NCORES):
        fsl = slice(c * FL, (c + 1) * FL)
        wnn16, wh8nn, wl8nn = _prep_gating_w(W_nn[fsl, :KU], SC)
        wno16, wh8no, wl8no = _prep_gating_w(W_no[fsl, :KU], SC)
        weh = np.ascontiguousarray(
            W_E[fsl, :KU].T.astype(np.float16)
            .reshape(8, 128, FL).transpose(1, 0, 2))
        ch_nn, cl_nn = cc_rows(W_nn, b_nn, fsl, SC)
        ch_no, cl_no = cc_rows(W_no, b_no, fsl, SC)
        ch_e, cl_e = cc_rows(W_E, b_E, fsl, 1.0)
        ccd = np.stack([np.concatenate([ch_nn, ch_no, ch_e]),
                        np.concatenate([cl_nn, cl_no, cl_e])]).astype(f32)
        im = {
            "xhT": xhT, "xl8T": xl8T, "x88T": x88T,
            "whnn16": wnn16, "wh8nn": wh8nn, "wl8nn": wl8nn,
            "whno16": wno16, "wh8no": wh8no, "wl8no": wl8no,
            "weh16": weh,
            "noise_c": np.ascontiguousarray(
                noise4[:, c * DL:(c + 1) * DL, :].reshape(S, FL)),
            "ccd": np.ascontiguousarray(ccd),
        }
        in_maps.append(im)

    nc = _get_program()
    res = run_bass_kernel_spmd(nc, in_maps, core_ids=list(range(NCORES)),
                               trace=TRACE)
    _CACHE["last_results"] = res
    out = np.empty((1, S, DIM), dtype=f32)
    for c in range(NCORES):
        out[0, :, c * DL:(c + 1) * DL] = res.results[c]["out_c"]
    return out


# revision 3
# speedup vs baseline: 1.9191x; 1.0504x over previous
"""Trainium2 Bass kernel for nn_Experts (topk_masking).

Math (reference):
  R = concat(h,us,ue) @ W_r.T + b_r                       [1,1,512]
  x = concat(u, R.broadcast)                              [1,S,1536]
  h1 = (x @ W_nn.T + b_nn).reshape(S,512,16)
  h2 = (x @ W_no.T + b_no).reshape(S,512,16) * noise
  g  = top2-masked softmax over experts of (h1+h2)
  e  = (x @ W_E.T + b_E).reshape(S,512,16)
  out = (g*e).mean(-1)                                    [1,S,512]

Sharding: the NE*DIM output-feature dim of the three projections is sharded
across 8 cores (64 dims x 16 experts each, contiguous feature slice). The
token-independent R-path is computed on the host in fp64 and folded into a
per-feature constant c[f]; the device contracts only over u's 1024 features.

Precision: gating logits accumulate in one PSUM chain at scale 2^18:
  fp16(x) @ fp16(W*2^18)                 main term (<=11-bit products, exact)
  e4m3(xl*2^14) @ e4m3(wA*2^4)           x-residual term, fp8 DoubleRow
  e4m3(x*2^3) @ e4m3(wl*2^15)            W-residual term, fp8 DoubleRow
  + c*2^18 via a K=2 f32r ones-matmul (trunc11 2-row split)
The 2^-18 descale folds into exp()'s scale operand; max/compare ops are
scale-invariant. This lands gating logits at ~2^-16 relative accuracy so
top-2 selection matches the fp32 reference. The e-matmul runs in fp16.
"""
import numpy as np
import ml_dtypes

DIM = 512
NE = 16
S = 4096
KU = 2 * DIM        # u features = 1024
NCORES = 8
DL = DIM // NCORES  # 64 dims per core
FL = DL * NE        # 1024 features per core
MCH = S // 128      # 32 token chunks

SC = 2.0 ** 18      # gating PSUM scale
ISC = 2.0 ** -18

_MASK11 = np.uint32(0xFFFFF000)  # keep 11 explicit mantissa bits

TRACE = False
_CACHE = {}


def _trunc11(a):
    a = np.ascontiguousarray(a, dtype=np.float32)
    return (a.view(np.uint32) & _MASK11).view(np.float32)


def _build():
    import concourse.bass as bass
    import concourse.mybir as mybir
    import concourse.tile as tile
    from concourse import bacc
    from contextlib import ExitStack

    F32 = mybir.dt.float32
    F32R = mybir.dt.float32r
    F16 = mybir.dt.float16
    F8 = mybir.dt.float8e4
    AX = mybir.AxisListType
    OP = mybir.AluOpType
    ACTF = mybir.ActivationFunctionType
    DR = mybir.MatmulPerfMode.DoubleRow

    nc = bacc.Bacc("TRN2", target_bir_lowering=False, debug=False,
                   num_devices=NCORES)

    def dram(name, shape, dt, kind="ExternalInput"):
        return nc.dram_tensor(name, shape, dt, kind=kind)

    # per-core inputs (same names on every core; data differs per core)
    xhT = dram("xhT", [128, 8, S], F16)           # fp16(u), k = kc*128+p
    xl8T = dram("xl8T", [128, 2, 4, S], F8)       # e4m3(xl*2^14), k=256j+128i+p
    x88T = dram("x88T", [128, 2, 4, S], F8)       # e4m3(u*2^3)
    noise_c = dram("noise_c", [S, FL], F32)
    whnn16 = dram("whnn16", [128, 8, FL], F16)    # fp16(W_nn.T*2^18)
    whno16 = dram("whno16", [128, 8, FL], F16)
    weh16 = dram("weh16", [128, 8, FL], F16)      # fp16(W_E.T)
    wh8nn = dram("wh8nn", [128, 2, 4, FL], F8)    # e4m3(wA_nn.T*2^4)
    wl8nn = dram("wl8nn", [128, 2, 4, FL], F8)    # e4m3(wl_nn.T*2^15)
    wh8no = dram("wh8no", [128, 2, 4, FL], F8)
    wl8no = dram("wl8no", [128, 2, 4, FL], F8)
    ccd = dram("ccd", [2, 3 * FL], F32R)          # trunc11 rows of c constants
    out_c = dram("out_c", [S, DL], F32, kind="ExternalOutput")

    with tile.TileContext(nc) as tc, ExitStack() as ctx:
        wpool = ctx.enter_context(tc.tile_pool(name="w", bufs=1))

        weh_t = wpool.tile([128, 8, FL], F16)
        whno_t = wpool.tile([128, 8, FL], F16)
        wh8no_t = wpool.tile([128, 2, 4, FL], F8)
        wl8no_t = wpool.tile([128, 2, 4, FL], F8)
        whnn_t = wpool.tile([128, 8, FL], F16)
        wh8nn_t = wpool.tile([128, 2, 4, FL], F8)
        wl8nn_t = wpool.tile([128, 2, 4, FL], F8)
        ccsb = wpool.tile([2, 3 * FL], F32R)

        onesf = wpool.tile([2, 128], F32)
        nc.vector.memset(onesf[:], 1.0)
        ones2 = wpool.tile([2, 128], F32R)
        nc.vector.tensor_copy(ones2[:], onesf[:])

        spool = ctx.enter_context(tc.tile_pool(name="stream", bufs=2))
        epool = ctx.enter_context(tc.tile_pool(name="epi", bufs=2))
        phpool = ctx.enter_context(tc.tile_pool(name="phps", bufs=1,
                                                space="PSUM"))
        pepool = ctx.enter_context(tc.tile_pool(name="peps", bufs=1,
                                                space="PSUM"))

        def fetch_x(m):
            tsl = slice(m * 128, (m + 1) * 128)
            xh_t = spool.tile([128, 8, 128], F16, tag="xh")
            xl8_t = spool.tile([128, 2, 4, 128], F8, tag="xl8")
            x88_t = spool.tile([128, 2, 4, 128], F8, tag="x88")
            nz_t = spool.tile([128, FL], F32, tag="nz")
            nc.sync.dma_start(xh_t[:], xhT.ap()[:, :, tsl])
            nc.sync.dma_start(xl8_t[:], xl8T.ap()[:, :, :, tsl])
            nc.sync.dma_start(x88_t[:], x88T.ap()[:, :, :, tsl])
            nc.sync.dma_start(nz_t[:], noise_c.ap()[tsl, :])
            return xh_t, xl8_t, x88_t, nz_t

        # weight DMAs split per-k and interleaved with the chunk-0/1 input
        # prefetch, ordered to match the PE's consumption order (h2, h1, e)
        # so chunk 0 can start ~2us in instead of waiting for all weights.
        xq = [fetch_x(0)]
        for k in range(8):
            nc.sync.dma_start(whno_t[:, k, :], whno16.ap()[:, k, :])
        nc.sync.dma_start(wh8no_t[:], wh8no.ap())
        nc.sync.dma_start(wl8no_t[:], wl8no.ap())
        nc.sync.dma_start(ccsb[:], ccd.ap())
        for k in range(8):
            nc.sync.dma_start(whnn_t[:, k, :], whnn16.ap()[:, k, :])
        nc.sync.dma_start(wh8nn_t[:], wh8nn.ap())
        nc.sync.dma_start(wl8nn_t[:], wl8nn.ap())
        xq.append(fetch_x(1))
        for k in range(8):
            nc.sync.dma_start(weh_t[:, k, :], weh16.ap()[:, k, :])

        def gating_phase(pt, xh_t, xl8_t, x88_t, w16_t, w8h_t, w8l_t, coff):
            for k in range(8):
                st = (k == 0)
                for half in range(2):
                    fsl = slice(half * 512, (half + 1) * 512)
                    nc.tensor.matmul(pt[:, fsl], xh_t[:, k, :],
                                     w16_t[:, k, fsl], start=st, stop=False)
            for j in range(4):
                for half in range(2):
                    fsl = slice(half * 512, (half + 1) * 512)
                    nc.tensor.matmul(pt[:, fsl], xl8_t[:, :, j, :],
                                     w8h_t[:, :, j, fsl],
                                     start=False, stop=False, perf_mode=DR)
                    nc.tensor.matmul(pt[:, fsl], x88_t[:, :, j, :],
                                     w8l_t[:, :, j, fsl],
                                     start=False, stop=False, perf_mode=DR)
            for half in range(2):
                fsl = slice(half * 512, (half + 1) * 512)
                nc.tensor.matmul(pt[:, fsl], ones2[:],
                                 ccsb[:, coff + half * 512:
                                      coff + (half + 1) * 512],
                                 start=False, stop=True)

        for m in range(MCH):
            tsl = slice(m * 128, (m + 1) * 128)
            xh_t, xl8_t, x88_t, nz_t = xq[m]
            if m + 2 < MCH:
                xq.append(fetch_x(m + 2))

            h1p = phpool.tile([128, FL], F32, tag="h1")
            h2p = phpool.tile([128, FL], F32, tag="h2")
            ep = pepool.tile([128, FL], F32, tag="e")

            # ---- h2 phase: fp16 mains + fp8 DoubleRow residuals -----------
            gating_phase(h2p, xh_t, xl8_t, x88_t, whno_t, wh8no_t, wl8no_t, FL)
            t_t = epool.tile([128, FL], F32, tag="t")
            nc.vector.tensor_mul(t_t[:], h2p[:], nz_t[:])

            # ---- h1 phase -------------------------------------------------
            gating_phase(h1p, xh_t, xl8_t, x88_t, whnn_t, wh8nn_t, wl8nn_t, 0)
            m_t = epool.tile([128, FL], F32, tag="m")
            nc.vector.tensor_add(m_t[:], t_t[:], h1p[:])

            # top-2 mask chain runs on DVE while the PE does the e phase
            mg = m_t[:].rearrange("p (d e) -> p d e", e=NE)
            v1 = epool.tile([128, DL], F32, tag="v1")
            nc.vector.tensor_reduce(v1[:], mg, AX.X, op=OP.max)
            eq1 = epool.tile([128, FL], F32, tag="eq1")
            nc.vector.tensor_tensor(eq1[:].rearrange("p (d e) -> p d e", e=NE),
                                    mg, v1[:].broadcast_to([128, DL, NE]),
                                    OP.is_equal)
            m2 = epool.tile([128, FL], F32, tag="m2")
            nc.vector.scalar_tensor_tensor(m2[:], eq1[:], -1e30, m_t[:],
                                           OP.mult, OP.add)
            v2 = epool.tile([128, DL], F32, tag="v2")
            nc.vector.tensor_reduce(v2[:], m2[:].rearrange("p (d e) -> p d e",
                                                           e=NE),
                                    AX.X, op=OP.max)
            mask = epool.tile([128, FL], F32, tag="mask")
            nc.vector.tensor_tensor(mask[:].rearrange("p (d e) -> p d e", e=NE),
                                    mg, v2[:].broadcast_to([128, DL, NE]),
                                    OP.is_ge)
            q = epool.tile([128, FL], F32, tag="q")
            nc.scalar.activation(q[:], m_t[:], ACTF.Exp, scale=ISC)

            # ---- e phase (fp16) -------------------------------------------
            for k in range(8):
                st = (k == 0)
                for half in range(2):
                    fsl = slice(half * 512, (half + 1) * 512)
                    nc.tensor.matmul(ep[:, fsl], xh_t[:, k, :],
                                     weh_t[:, k, fsl], start=st, stop=False)
            for half in range(2):
                fsl = slice(half * 512, (half + 1) * 512)
                nc.tensor.matmul(ep[:, fsl], ones2[:],
                                 ccsb[:, 2 * FL + half * 512:
                                      2 * FL + (half + 1) * 512],
                                 start=False, stop=True)

            t1 = epool.tile([128, FL], F32, tag="t1")
            nc.vector.tensor_mul(t1[:], mask[:], ep[:])
            t2 = epool.tile([128, FL], F32, tag="t2")
            nc.vector.tensor_mul(t2[:], t1[:], q[:])
            s_t = epool.tile([128, DL], F32, tag="s")
            nc.vector.tensor_reduce(s_t[:],
                                    t2[:].rearrange("p (d e) -> p d e", e=NE),
                                    AX.X, op=OP.add)

            ev12 = epool.tile([128, 2 * DL], F32, tag="ev12")
            nc.scalar.activation(ev12[:, :DL], v1[:], ACTF.Exp, scale=ISC)
            nc.scalar.activation(ev12[:, DL:], v2[:], ACTF.Exp, scale=ISC)
            z_t = epool.tile([128, DL], F32, tag="z")
            nc.vector.tensor_add(z_t[:], ev12[:, :DL], ev12[:, DL:])
            r_t = epool.tile([128, DL], F32, tag="r")
            nc.vector.reciprocal(r_t[:], z_t[:])
            o_t = epool.tile([128, DL], F32, tag="o")
            nc.vector.scalar_tensor_tensor(o_t[:], s_t[:], 1.0 / NE, r_t[:],
                                           OP.mult, OP.mult)
            nc.sync.dma_start(out_c.ap()[tsl, :], o_t[:])

    nc.compile()
    return nc


def _get_program():
    if "nc" not in _CACHE:
        _CACHE["nc"] = _build()
    return _CACHE["nc"]


def _prep_shared(u):
    f32 = np.float32
    E4 = ml_dtypes.float8_e4m3
    u2 = np.ascontiguousarray(np.asarray(u, dtype=f32).reshape(S, KU))
    xh16 = u2.astype(np.float16)                      # [S, K]
    xl = (u2 - xh16.astype(f32)).astype(f32)

    # main layout [p, kc, t]: k = kc*128 + p
    xhT = np.ascontiguousarray(xh16.T.reshape(8, 128, S).transpose(1, 0, 2))
    # DoubleRow layout [p, i, j, t]: k = j*256 + i*128 + p
    xl8T = np.ascontiguousarray(
        (xl.T * f32(2.0 ** 14)).astype(E4)
        .reshape(4, 2, 128, S).transpose(2, 1, 0, 3))
    x88T = np.ascontiguousarray(
        (u2.T * f32(2.0 ** 3)).astype(E4)
        .reshape(4, 2, 128, S).transpose(2, 1, 0, 3))
    return u2, xhT, xl8T, x88T


def _prep_gating_w(Wu, scale):
    # Wu: [FL, KU] fp32 feature-slice of a gating projection (u-part)
    f32 = np.float32
    E4 = ml_dtypes.float8_e4m3
    WuT = np.ascontiguousarray(Wu.T.astype(f32))      # [K, F]
    wh16 = (WuT * f32(scale)).astype(np.float16)
    wA = (wh16.astype(f32) * f32(1.0 / scale)).astype(f32)
    wl = (WuT - wA).astype(f32)
    w16 = np.ascontiguousarray(wh16.reshape(8, 128, FL).transpose(1, 0, 2))
    wh8 = np.ascontiguousarray(
        (wA * f32(2.0 ** 4)).astype(E4)
        .reshape(4, 2, 128, FL).transpose(2, 1, 0, 3))
    wl8 = np.ascontiguousarray(
        (wl * f32(2.0 ** 15)).astype(E4)
        .reshape(4, 2, 128, FL).transpose(2, 1, 0, 3))
    return w16, wh8, wl8


def kernel(h, us, ue, u, noise, W_nn, b_nn, W_no, b_no, W_E, b_E, W_r, b_r):
    from concourse.bass_utils import run_bass_kernel_spmd

    f32 = np.float32
    u2, xhT, xl8T, x88T = _prep_shared(u)

    # host R-path in fp64 (token-independent, ~4 MFLOP)
    hx = np.concatenate([np.asarray(h, dtype=f32).ravel(),
                         np.asarray(us, dtype=f32).ravel(),
                         np.asarray(ue, dtype=f32).ravel()]).astype(np.float64)
    R = hx @ np.asarray(W_r, dtype=np.float64).T + np.asarray(
        b_r, dtype=np.float64)                        # [512]

    W_nn = np.asarray(W_nn, dtype=f32)
    W_no = np.asarray(W_no, dtype=f32)
    W_E = np.asarray(W_E, dtype=f32)
    noise4 = np.asarray(noise, dtype=f32).reshape(S, DIM, NE)

    def cc_rows(W, b, fsl, scale):
        c = (np.asarray(b, np.float64)[fsl]
             + R @ np.asarray(W, np.float64)[fsl, KU:].T) * scale
        c32 = c.astype(f32)
        ch = _trunc11(c32)
        cl = _trunc11((c32.astype(np.float64) - ch).astype(f32))
        return ch, cl

    in_maps = []
    for c in range(NCORES):
        fsl = slice(c * FL, (c + 1) * FL)
        wnn16, wh8nn, wl8nn = _prep_gating_w(W_nn[fsl, :KU], SC)
        wno16, wh8no, wl8no = _prep_gating_w(W_no[fsl, :KU], SC)
        weh = np.ascontiguousarray(
            W_E[fsl, :KU].T.astype(np.float16)
            .reshape(8, 128, FL).transpose(1, 0, 2))
        ch_nn, cl_nn = cc_rows(W_nn, b_nn, fsl, SC)
        ch_no, cl_no = cc_rows(W_no, b_no, fsl, SC)
        ch_e, cl_e = cc_rows(W_E, b_E, fsl, 1.0)
        ccd = np.stack([np.concatenate([ch_nn, ch_no, ch_e]),
                        np.concatenate([cl_nn, cl_no, cl_e])]).astype(f32)
        im = {
            "xhT": xhT, "xl8T": xl8T, "x88T": x88T,
            "whnn16": wnn16, "wh8nn": wh8nn, "wl8nn": wl8nn,
            "whno16": wno16, "wh8no": wh8no, "wl8no": wl8no,
            "weh16": weh,
            "noise_c": np.ascontiguousarray(
                noise4[:, c * DL:(c + 1) * DL, :].reshape(S, FL)),
            "ccd": np.ascontiguousarray(ccd),
        }
        in_maps.append(im)

    nc = _get_program()
    res = run_bass_kernel_spmd(nc, in_maps, core_ids=list(range(NCORES)),
                               trace=TRACE)
    _CACHE["last_results"] = res
    out = np.empty((1, S, DIM), dtype=f32)
    for c in range(NCORES):
        out[0, :, c * DL:(c + 1) * DL] = res.results[c]["out_c"]
    return out
